# revision 1
# baseline (speedup 1.0000x reference)
"""Trainium2 Bass kernel for nn_DecoderLayer_19816979104174.

Data-parallel over batch: each of the 8 NeuronCores runs one batch element's
full decoder layer. All matmuls in bf16 (fp32 PSUM accumulation). Attention is
computed in transposed [s, t] layout so that:
  - Q/K/V projections consume a single on-chip transpose of x,
  - softmax row-sums come from ones-column matmuls on the PE,
  - the attention-weighted sums feed the output projection with no transposes.
Causal structure is exploited by never computing s>t blocks (the exp buffer is
zero-initialized once; zeros persist across heads). The output projection is
accumulated per-head into an SBUF fp32 accumulator to bound SBUF usage.
"""

import sys

sys.path.insert(0, "/opt/trn_rl_repo")
sys.path.insert(0, "/root/.axon_site/_ro/trn_rl_repo")

import numpy as np

B, T, S, D, H, F = 8, 1024, 1024, 512, 8, 2048
P = 128
NT, ND, NS, NF = T // P, D // P, S // P, F // P
NC2 = T // 512  # 512-wide t chunks
SCALE = 1.0 / float(np.sqrt(D))
LN_EPS = 1e-5

_CACHE = {}


def _build():
    if "nc" in _CACHE:
        return _CACHE["nc"]

    import concourse.tile as tile
    import concourse.mybir as mybir
    from concourse import bacc
    from concourse.masks import make_identity
    from contextlib import ExitStack

    bf16 = mybir.dt.bfloat16
    f32 = mybir.dt.float32
    AF = mybir.ActivationFunctionType
    OP = mybir.AluOpType

    nc = bacc.Bacc("TRN2")

    # ---- DRAM I/O -----------------------------------------------------
    d_x = nc.dram_tensor("x32", [T, D], f32, kind="ExternalInput")
    d_wq = nc.dram_tensor("wq", [H, D, D], bf16, kind="ExternalInput")
    d_wk = nc.dram_tensor("wk", [H, D, D], bf16, kind="ExternalInput")
    d_wv = nc.dram_tensor("wv", [H, D, D], bf16, kind="ExternalInput")
    d_wqm = nc.dram_tensor("wqm", [H, D, D], bf16, kind="ExternalInput")
    d_wo = nc.dram_tensor("wo", [H * D, D], bf16, kind="ExternalInput")
    d_wom = nc.dram_tensor("wom", [H * D, D], bf16, kind="ExternalInput")
    d_w1 = nc.dram_tensor("w1", [D, F], bf16, kind="ExternalInput")
    d_w2 = nc.dram_tensor("w2", [F, D], bf16, kind="ExternalInput")
    d_bq = nc.dram_tensor("bq_c", [P, H * ND], f32, kind="ExternalInput")
    d_bk = nc.dram_tensor("bk_c", [P, H * ND], f32, kind="ExternalInput")
    d_bqm = nc.dram_tensor("bqm_c", [P, H * ND], f32, kind="ExternalInput")
    d_b1 = nc.dram_tensor("b1_c", [P, NF], f32, kind="ExternalInput")
    d_bo = nc.dram_tensor("bo_row", [1, D], bf16, kind="ExternalInput")
    d_bom = nc.dram_tensor("bom_row", [1, D], bf16, kind="ExternalInput")
    d_b2 = nc.dram_tensor("b2_row", [1, D], bf16, kind="ExternalInput")
    d_memk = nc.dram_tensor("memk", [S, D], bf16, kind="ExternalInput")
    d_memv = nc.dram_tensor("memv", [S, D], bf16, kind="ExternalInput")
    d_tpad = nc.dram_tensor("tpad", [P, NS], f32, kind="ExternalInput")
    d_spad = nc.dram_tensor("spad", [P, NS], f32, kind="ExternalInput")
    d_diag = nc.dram_tensor("diag", [P, P], f32, kind="ExternalInput")
    d_out = nc.dram_tensor("out", [T, D], f32, kind="ExternalOutput")

    with tile.TileContext(nc) as tc, ExitStack() as ctx:
        const = ctx.enter_context(tc.tile_pool(name="const", bufs=1))
        small = ctx.enter_context(tc.tile_pool(name="small", bufs=2))
        psum_mm = ctx.enter_context(tc.tile_pool(name="psum_mm", bufs=4, space="PSUM"))
        psum_tr = ctx.enter_context(tc.tile_pool(name="psum_tr", bufs=2, space="PSUM"))
        psum_rs = ctx.enter_context(tc.tile_pool(name="psum_rs", bufs=2, space="PSUM"))

        # ---- constants / small inputs --------------------------------
        ident_b = const.tile([P, P], bf16)
        make_identity(nc, ident_b)
        ident_f = const.tile([P, P], f32)
        make_identity(nc, ident_f)
        ones_col = const.tile([P, 1], bf16)
        nc.vector.memset(ones_col[:], 1.0)
        ones_row = const.tile([1, P], bf16)
        nc.vector.memset(ones_row[:], 1.0)
        eps_t = const.tile([P, 1], f32)
        nc.vector.memset(eps_t[:], LN_EPS)
        diag_sb = const.tile([P, P], f32)
        nc.gpsimd.dma_start(out=diag_sb[:], in_=d_diag.ap())
        tpad_sb = const.tile([P, NS], f32)
        nc.gpsimd.dma_start(out=tpad_sb[:], in_=d_tpad.ap())
        spad_sb = const.tile([P, NS], f32)
        nc.gpsimd.dma_start(out=spad_sb[:], in_=d_spad.ap())
        bq_sb = const.tile([P, H * ND], f32)
        nc.gpsimd.dma_start(out=bq_sb[:], in_=d_bq.ap())
        bk_sb = const.tile([P, H * ND], f32)
        nc.gpsimd.dma_start(out=bk_sb[:], in_=d_bk.ap())
        bqm_sb = const.tile([P, H * ND], f32)
        nc.gpsimd.dma_start(out=bqm_sb[:], in_=d_bqm.ap())
        b1_sb = const.tile([P, NF], f32)
        nc.gpsimd.dma_start(out=b1_sb[:], in_=d_b1.ap())
        bo_sb = const.tile([1, D], bf16)
        nc.gpsimd.dma_start(out=bo_sb[:], in_=d_bo.ap())
        bom_sb = const.tile([1, D], bf16)
        nc.gpsimd.dma_start(out=bom_sb[:], in_=d_bom.ap())
        b2_sb = const.tile([1, D], bf16)
        nc.gpsimd.dma_start(out=b2_sb[:], in_=d_b2.ap())

        # ---- pools with phase-scoped lifetimes (LIFO close order) ----
        es_x2 = ExitStack()     # x2/x2T: phases 4-5
        x2_pool = es_x2.enter_context(tc.tile_pool(name="x2p", bufs=1))
        es_attn = ExitStack()   # expT + acc: phases 1-4
        attn_pool = es_attn.enter_context(tc.tile_pool(name="attn", bufs=1))
        es_x1 = ExitStack()     # x1/x1T: phases 2-4
        x1_pool = es_x1.enter_context(tc.tile_pool(name="x1p", bufs=1))
        es_x32 = ExitStack()    # x resident: phases 0-2
        x32_pool = es_x32.enter_context(tc.tile_pool(name="x32p", bufs=1))

        expT = attn_pool.tile([P, NS, T], bf16, tag="expT")
        nc.gpsimd.memset(expT[:], 0.0)
        acc_sb = attn_pool.tile([P, NT, D], f32, tag="acc")
        x32_sb = x32_pool.tile([P, NT, D], f32)
        for tb in range(NT):
            nc.sync.dma_start(
                out=x32_sb[:, tb, :],
                in_=d_x.ap().rearrange("(tb p) d -> p tb d", p=P)[:, tb, :])

        def transpose_to(src_ap, dstT, ident, dt_blocks, tb, dtype_ps):
            for dt in range(dt_blocks):
                tr_ps = psum_tr.tile([P, P], dtype_ps, tag="tr")
                nc.tensor.transpose(
                    tr_ps[:], src_ap[:, dt * P:(dt + 1) * P], ident[:])
                nc.vector.tensor_copy(dstT[:, dt, tb * P:(tb + 1) * P], tr_ps[:])

        def layernorm(src_ap, resid_ap, dst_ap):
            res = small.tile([P, D], f32, tag="ln_res")
            nc.vector.tensor_tensor(out=res[:], in0=src_ap, in1=resid_ap, op=OP.add)
            stats = small.tile([P, 6], f32, tag="ln_stats")
            nc.vector.bn_stats(stats[:], res[:])
            mv = small.tile([P, 2], f32, tag="ln_mv")
            nc.vector.bn_aggr(mv[:], stats[:])
            std = small.tile([P, 1], f32, tag="ln_std")
            nc.scalar.activation(std[:], mv[:, 1:2], AF.Sqrt, bias=eps_t[:])
            istd = small.tile([P, 1], f32, tag="ln_istd")
            nc.vector.reciprocal(istd[:], std[:])
            nc.vector.tensor_scalar(
                out=dst_ap, in0=res[:], scalar1=mv[:, 0:1], scalar2=istd[:],
                op0=OP.subtract, op1=OP.mult)

        def attention(qT, kT_ap, v_ap, pad_sb, causal, rbc_pool, hoT_pool):
            """softmax(scale * kT.T-x-qT + pad) -> hoT [e, t]; returns hoT."""
            recipT = rbc_pool.tile([1, T], f32, tag="recipT")
            recip_bc = rbc_pool.tile([P, T], f32, tag="recip_bc")

            def rowsum_chunk(c):
                # emit as soon as the last s-block feeding chunk c is exp'd,
                # so the recip/broadcast chain hides behind later PE work
                jmax = min(4 * (c + 1), NS) if causal else NS
                rs_ps = psum_rs.tile([1, 512], f32, tag="rs")
                for j in range(jmax):
                    nc.tensor.matmul(
                        rs_ps[:], lhsT=ones_col[:],
                        rhs=expT[:, j, c * 512:(c + 1) * 512],
                        start=(j == 0), stop=(j == jmax - 1))
                sl = slice(c * 512, (c + 1) * 512)
                nc.vector.reciprocal(recipT[:, sl], rs_ps[:])
                nc.gpsimd.partition_broadcast(recip_bc[:, sl], recipT[:, sl])

            for j in range(NS):
                c_lo = (j * P) // 512 if causal else 0
                for c in range(c_lo, NC2):
                    lo = max(j * P, c * 512) if causal else c * 512
                    w = (c + 1) * 512 - lo
                    att_ps = psum_mm.tile([P, 512], f32, tag="mm")
                    for et in range(ND):
                        nc.tensor.matmul(
                            att_ps[:, :w],
                            lhsT=kT_ap[:, et, j * P:(j + 1) * P],
                            rhs=qT[:, et, lo:(c + 1) * 512],
                            start=(et == 0), stop=(et == ND - 1))
                    if causal and lo == j * P:
                        nc.vector.tensor_tensor(
                            out=att_ps[:, 0:P], in0=att_ps[:, 0:P],
                            in1=diag_sb[:], op=OP.add)
                    nc.scalar.activation(
                        expT[:, j, lo:(c + 1) * 512], att_ps[:, :w], AF.Exp,
                        bias=pad_sb[:, j:j + 1], scale=SCALE)
                if causal and j == 3:
                    rowsum_chunk(0)
            if causal:
                rowsum_chunk(1)
            else:
                rowsum_chunk(0)
                rowsum_chunk(1)
            hoT = hoT_pool.tile([P, ND, T], bf16, tag="hoT")
            for eb in range(ND):
                for c in range(NC2):
                    jmax = min(4 * (c + 1), NS) if causal else NS
                    ho_ps = psum_mm.tile([P, 512], f32, tag="mm")
                    for j in range(jmax):
                        nc.tensor.matmul(
                            ho_ps[:],
                            lhsT=v_ap[:, j, eb * P:(eb + 1) * P],
                            rhs=expT[:, j, c * 512:(c + 1) * 512],
                            start=(j == 0), stop=(j == jmax - 1))
                    nc.vector.tensor_tensor(
                        out=hoT[:, eb, c * 512:(c + 1) * 512],
                        in0=ho_ps[:], in1=recip_bc[:, c * 512:(c + 1) * 512],
                        op=OP.mult)
            return hoT

        def oproj_partial(h, hoT, woh, brow_sb):
            """acc_sb (+)= hoT.T @ w[h-block] (+ bias row on h==0)."""
            for tb in range(NT):
                sa_ps = psum_mm.tile([P, 512], f32, tag="mm")
                for kt in range(ND):
                    nc.tensor.matmul(
                        sa_ps[:],
                        lhsT=hoT[:, kt, tb * P:(tb + 1) * P],
                        rhs=woh[:, kt, :],
                        start=(kt == 0), stop=(h != 0 and kt == ND - 1))
                if h == 0:
                    nc.tensor.matmul(
                        sa_ps[:], lhsT=ones_row[:, 0:P], rhs=brow_sb[:],
                        start=False, stop=True)
                    nc.vector.tensor_copy(acc_sb[:, tb, :], sa_ps[:])
                else:
                    nc.vector.tensor_tensor(
                        out=acc_sb[:, tb, :], in0=acc_sb[:, tb, :],
                        in1=sa_ps[:], op=OP.add)

        # ============ phase 0+1: xT, self attention ===================
        with tc.tile_pool(name="xT", bufs=1) as xT_pool, \
             tc.tile_pool(name="qkv", bufs=1) as qkv_pool, \
             tc.tile_pool(name="hoTp", bufs=2) as hoT_pool, \
             tc.tile_pool(name="wstream", bufs=2) as wstream, \
             tc.tile_pool(name="rbc", bufs=2) as rbc_pool:
            xT = xT_pool.tile([P, ND, T], bf16)
            for tb in range(NT):
                transpose_to(x32_sb[:, tb, :], xT, ident_f, ND, tb, f32)
            qT = qkv_pool.tile([P, ND, T], bf16, tag="qT")
            kT = qkv_pool.tile([P, ND, T], bf16, tag="kT")
            v_sb = qkv_pool.tile([P, NS, D], bf16, tag="v")
            for h in range(H):
                wq_t = wstream.tile([P, ND, ND, P], bf16, tag="wq")
                nc.sync.dma_start(out=wq_t[:], in_=d_wq.ap()[h].rearrange(
                    "(kt p) (eb e) -> p kt eb e", p=P, e=P))
                wk_t = wstream.tile([P, ND, ND, P], bf16, tag="wk")
                nc.sync.dma_start(out=wk_t[:], in_=d_wk.ap()[h].rearrange(
                    "(kt p) (eb e) -> p kt eb e", p=P, e=P))
                wv_t = wstream.tile([P, ND, D], bf16, tag="wv")
                nc.sync.dma_start(out=wv_t[:], in_=d_wv.ap()[h].rearrange(
                    "(kt p) e -> p kt e", p=P))
                woh_t = wstream.tile([P, ND, D], bf16, tag="woh")
                nc.sync.dma_start(
                    out=woh_t[:],
                    in_=d_wo.ap()[h * D:(h + 1) * D].rearrange("(kt p) d -> p kt d", p=P))
                for eb in range(ND):
                    for c in range(NC2):
                        q_ps = psum_mm.tile([P, 512], f32, tag="mm")
                        for kt in range(ND):
                            nc.tensor.matmul(
                                q_ps[:], lhsT=wq_t[:, kt, eb, :],
                                rhs=xT[:, kt, c * 512:(c + 1) * 512],
                                start=(kt == 0), stop=(kt == ND - 1))
                        nc.vector.tensor_scalar_add(
                            qT[:, eb, c * 512:(c + 1) * 512], q_ps[:],
                            bq_sb[:, h * ND + eb:h * ND + eb + 1])
                        k_ps = psum_mm.tile([P, 512], f32, tag="mm")
                        for kt in range(ND):
                            nc.tensor.matmul(
                                k_ps[:], lhsT=wk_t[:, kt, eb, :],
                                rhs=xT[:, kt, c * 512:(c + 1) * 512],
                                start=(kt == 0), stop=(kt == ND - 1))
                        nc.vector.tensor_scalar_add(
                            kT[:, eb, c * 512:(c + 1) * 512], k_ps[:],
                            bk_sb[:, h * ND + eb:h * ND + eb + 1])
                for sb_ in range(NS):
                    v_ps = psum_mm.tile([P, 512], f32, tag="mm")
                    for kt in range(ND):
                        nc.tensor.matmul(
                            v_ps[:], lhsT=xT[:, kt, sb_ * P:(sb_ + 1) * P],
                            rhs=wv_t[:, kt, :],
                            start=(kt == 0), stop=(kt == ND - 1))
                    nc.vector.tensor_copy(v_sb[:, sb_, :], v_ps[:])
                hoT = attention(qT, kT, v_sb, tpad_sb, True, rbc_pool, hoT_pool)
                if h > 0:
                    oproj_partial(h - 1, prev[0], prev[1], bo_sb)
                prev = (hoT, woh_t)
            oproj_partial(H - 1, prev[0], prev[1], bo_sb)

        # ============ phase 2+3: LN1 -> x1/x1T, cross attention =======
        with tc.tile_pool(name="mem", bufs=1) as mem_pool, \
             tc.tile_pool(name="qkv2", bufs=1) as qkv2_pool, \
             tc.tile_pool(name="hoTp2", bufs=2) as hoT2_pool, \
             tc.tile_pool(name="wstream2", bufs=2) as wstream2, \
             tc.tile_pool(name="rbc2", bufs=2) as rbc2_pool:
            memk_sb = mem_pool.tile([P, NS, D], bf16, tag="memk")
            nc.sync.dma_start(out=memk_sb[:], in_=d_memk.ap().rearrange(
                "(st p) e -> p st e", p=P))
            memv_sb = mem_pool.tile([P, NS, D], bf16, tag="memv")
            nc.sync.dma_start(out=memv_sb[:], in_=d_memv.ap().rearrange(
                "(st p) e -> p st e", p=P))
            x1_sb = x1_pool.tile([P, NT, D], f32, tag="x1")
            x1T_sb = x1_pool.tile([P, ND, T], bf16, tag="x1T")
            memkT = mem_pool.tile([P, ND, S], bf16, tag="memkT")
            for tb in range(NT):
                layernorm(acc_sb[:, tb, :], x32_sb[:, tb, :], x1_sb[:, tb, :])
                transpose_to(x1_sb[:, tb, :], x1T_sb, ident_f, ND, tb, f32)
                # independent PE filler while the LN chain drains
                transpose_to(memk_sb[:, tb, :], memkT, ident_b, ND, tb, bf16)
            qmT = qkv2_pool.tile([P, ND, T], bf16, tag="qmT")
            for h in range(H):
                wqm_t = wstream2.tile([P, ND, ND, P], bf16, tag="wqm")
                nc.sync.dma_start(out=wqm_t[:], in_=d_wqm.ap()[h].rearrange(
                    "(kt p) (eb e) -> p kt eb e", p=P, e=P))
                womh_t = wstream2.tile([P, ND, D], bf16, tag="womh")
                nc.sync.dma_start(
                    out=womh_t[:],
                    in_=d_wom.ap()[h * D:(h + 1) * D].rearrange("(kt p) d -> p kt d", p=P))
                for eb in range(ND):
                    for c in range(NC2):
                        q_ps = psum_mm.tile([P, 512], f32, tag="mm")
                        for kt in range(ND):
                            nc.tensor.matmul(
                                q_ps[:], lhsT=wqm_t[:, kt, eb, :],
                                rhs=x1T_sb[:, kt, c * 512:(c + 1) * 512],
                                start=(kt == 0), stop=(kt == ND - 1))
                        nc.vector.tensor_scalar_add(
                            qmT[:, eb, c * 512:(c + 1) * 512], q_ps[:],
                            bqm_sb[:, h * ND + eb:h * ND + eb + 1])
                hoT = attention(qmT, memkT, memv_sb, spad_sb, False,
                                rbc2_pool, hoT2_pool)
                if h > 0:
                    oproj_partial(h - 1, prev[0], prev[1], bom_sb)
                prev = (hoT, womh_t)
            oproj_partial(H - 1, prev[0], prev[1], bom_sb)
        es_x32.close()

        # ============ phase 4+5: LN2 -> x2/x2T, FFN + LN3 =============
        with tc.tile_pool(name="ffn", bufs=1) as ffn_pool:
            w1_t = ffn_pool.tile([P, ND, NF, P], bf16, tag="w1")
            nc.sync.dma_start(out=w1_t[:], in_=d_w1.ap().rearrange(
                "(kt p) (fb f) -> p kt fb f", p=P, f=P))
            w2_t = ffn_pool.tile([P, NF, D], bf16, tag="w2")
            nc.sync.dma_start(out=w2_t[:], in_=d_w2.ap().rearrange(
                "(kt p) d -> p kt d", p=P))
            f1T = ffn_pool.tile([P, NF, T], bf16, tag="f1T")
            x2_sb = x2_pool.tile([P, NT, D], f32, tag="x2")
            x2T_sb = x2_pool.tile([P, ND, T], bf16, tag="x2T")

            def f1_chunk(c):
                for fb in range(NF):
                    f_ps = psum_mm.tile([P, 512], f32, tag="mm")
                    for kt in range(ND):
                        nc.tensor.matmul(
                            f_ps[:], lhsT=w1_t[:, kt, fb, :],
                            rhs=x2T_sb[:, kt, c * 512:(c + 1) * 512],
                            start=(kt == 0), stop=(kt == ND - 1))
                    nc.scalar.activation(
                        f1T[:, fb, c * 512:(c + 1) * 512], f_ps[:], AF.Relu,
                        bias=b1_sb[:, fb:fb + 1])

            for tb in range(NT):
                layernorm(acc_sb[:, tb, :], x1_sb[:, tb, :], x2_sb[:, tb, :])
                transpose_to(x2_sb[:, tb, :], x2T_sb, ident_f, ND, tb, f32)
                # start FFN chunk as soon as the t-blocks feeding it are done
                if tb == 3:
                    f1_chunk(0)
            f1_chunk(1)
            for tb in range(NT):
                o_ps = psum_mm.tile([P, 512], f32, tag="mm")
                for kt in range(NF):
                    nc.tensor.matmul(
                        o_ps[:], lhsT=f1T[:, kt, tb * P:(tb + 1) * P],
                        rhs=w2_t[:, kt, :], start=(kt == 0), stop=False)
                nc.tensor.matmul(
                    o_ps[:], lhsT=ones_row[:, 0:P], rhs=b2_sb[:],
                    start=False, stop=True)
                out_sb = small.tile([P, D], f32, tag="out_sb")
                layernorm(o_ps[:], x2_sb[:, tb, :], out_sb[:])
                nc.sync.dma_start(
                    out=d_out.ap().rearrange("(tb p) d -> p tb d", p=P)[:, tb, :],
                    in_=out_sb[:])
        es_x1.close()
        es_attn.close()
        es_x2.close()

    nc.compile()
    _CACHE["nc"] = nc
    return nc


def make_in_maps(inputs):
    import ml_dtypes

    bf = ml_dtypes.bfloat16
    f32 = np.float32

    def col_layout(bias_hd):  # [H, D] -> [P, H*ND], col h*ND+eb
        return np.ascontiguousarray(
            bias_hd.reshape(H, ND, P).transpose(2, 0, 1).reshape(P, H * ND)
        ).astype(f32)

    wo_f = np.asarray(inputs["Wo_self"], f32)
    bo_row = np.asarray(inputs["bo_self"], f32).copy()
    bv = np.asarray(inputs["bv_self"], f32)
    for h in range(H):
        bo_row += bv[h] @ wo_f[h * D:(h + 1) * D]

    shared = {
        "wq": np.asarray(inputs["Wq_self"], f32).astype(bf),
        "wk": np.asarray(inputs["Wk_self"], f32).astype(bf),
        "wv": np.asarray(inputs["Wv_self"], f32).astype(bf),
        "wqm": np.asarray(inputs["Wq_mem"], f32).astype(bf),
        "wo": wo_f.astype(bf),
        "wom": np.asarray(inputs["Wo_mem"], f32).astype(bf),
        "w1": np.asarray(inputs["W1"], f32).astype(bf),
        "w2": np.asarray(inputs["W2"], f32).astype(bf),
        "bq_c": col_layout(np.asarray(inputs["bq_self"], f32)),
        "bk_c": col_layout(np.asarray(inputs["bk_self"], f32)),
        "bqm_c": col_layout(np.asarray(inputs["bq_mem"], f32)),
        "b1_c": np.ascontiguousarray(
            np.asarray(inputs["b1"], f32).reshape(NF, P).T).astype(f32),
        "bo_row": bo_row.reshape(1, D).astype(bf),
        "bom_row": np.asarray(inputs["bo_mem"], f32).reshape(1, D).astype(bf),
        "b2_row": np.asarray(inputs["b2"], f32).reshape(1, D).astype(bf),
        # attT is [s, t]: transpose the causal diagonal block
        "diag": np.ascontiguousarray(
            np.asarray(inputs["tgt_subsq_mask"], f32)[:P, :P].T),
    }
    in_maps = []
    for b in range(B):
        m = dict(shared)
        m["x32"] = np.ascontiguousarray(np.asarray(inputs["x"], f32)[b])
        m["memk"] = np.asarray(inputs["mem_keys"], f32)[b].astype(bf)
        m["memv"] = np.asarray(inputs["mem_values"], f32)[b].astype(bf)
        m["tpad"] = np.ascontiguousarray(
            np.asarray(inputs["tgt_padding_mask"], f32)[b, :, 0].reshape(NS, P).T)
        m["spad"] = np.ascontiguousarray(
            np.asarray(inputs["src_padding_mask"], f32)[b, :, 0].reshape(NS, P).T)
        in_maps.append(m)
    return in_maps


def kernel(**inputs):
    from concourse.bass_utils import run_bass_kernel_spmd

    nc = _build()
    in_maps = make_in_maps(inputs)
    res = run_bass_kernel_spmd(nc, in_maps, list(range(B)))
    out = np.stack([np.asarray(res.results[i]["out"]) for i in range(B)])
    return out.astype(np.float32)



# revision 6
# speedup vs baseline: 1.9374x; 1.9374x over previous
"""Trainium2 Bass kernel for nn_DecoderLayer_19816979104174.

Data-parallel over batch: each of the 8 NeuronCores runs one batch element's
full decoder layer. Attention matmuls run in fp8e4m3 DoubleRow mode (K=256
per instruction, 2x bf16 throughput); the FFN stays bf16. Algebraic fusions
remove entire projections:
  - self-attn: k-projection folded into q via M = Wq@Wk^T (the k-bias drops
    exactly: it shifts all logits of a query equally and softmax is
    shift-invariant); v-projection and output projection fused via
    U_h = Wv_h@Wo_h (the v-bias rides softmax rowsum=1 into a folded bias row).
  - cross-attn: host-precomputed K~_h = Wqm_h@memk^T (per-key bias r[s] folded
    into the exp bias column) and V~_h = memv@Wom_h + bo_mem/H eliminate both
    the q projection and the output projection.
fp8 tensors are pre-scaled into e4m3's normal range; compensations ride the
exp() scale argument and single-pass scalar_tensor_tensor evacuations
(out = psum*alpha + residual).
"""

import sys

sys.path.insert(0, "/opt/trn_rl_repo")
sys.path.insert(0, "/root/.axon_site/_ro/trn_rl_repo")

import numpy as np

B, T, S, D, H, F = 8, 1024, 1024, 512, 8, 2048
P = 128
NT, ND, NS, NF = T // P, D // P, S // P, F // P
NC2 = T // 512
SCALE = 1.0 / float(np.sqrt(D))
LN_EPS = 1e-5

G_M = 64.0             # M = Wq@Wk^T stored as M*G_M
V_S = 1.0 / 16.0       # self rowsum ones value -> ctx8 = 16*ctx
G_U = 128.0            # U stored as U*G_U
A_SELF = 1.0 / (16.0 * 128.0)  # self oproj psum descale

_CACHE = {}


def _build():
    if "nc" in _CACHE:
        return _CACHE["nc"]

    import concourse.tile as tile
    import concourse.mybir as mybir
    from concourse import bacc
    from concourse.masks import make_identity
    from contextlib import ExitStack

    bf16 = mybir.dt.bfloat16
    f32 = mybir.dt.float32
    fp8 = mybir.dt.float8e4
    AF = mybir.ActivationFunctionType
    OP = mybir.AluOpType
    PM = mybir.MatmulPerfMode

    nc = bacc.Bacc("TRN2")

    d_x = nc.dram_tensor("x32", [T, D], f32, kind="ExternalInput")
    d_m8 = nc.dram_tensor("m8", [H, P, 2, 2, ND, P], fp8, kind="ExternalInput")
    d_u8 = nc.dram_tensor("u8", [P, 2, 2, H, D], fp8, kind="ExternalInput")
    d_kt8 = nc.dram_tensor("kt8", [H, P, 2, 2, S], fp8, kind="ExternalInput")
    d_vt8 = nc.dram_tensor("vt8", [H, P, NS // 2, 2, D], fp8, kind="ExternalInput")
    d_w1 = nc.dram_tensor("w1", [D, F], bf16, kind="ExternalInput")
    d_w2 = nc.dram_tensor("w2", [F, D], bf16, kind="ExternalInput")
    d_bq = nc.dram_tensor("bq_c", [P, H * ND], f32, kind="ExternalInput")
    d_b1 = nc.dram_tensor("b1_c", [P, NF], f32, kind="ExternalInput")
    d_bo = nc.dram_tensor("bo_row", [1, D], bf16, kind="ExternalInput")
    d_b2 = nc.dram_tensor("b2_row", [1, D], bf16, kind="ExternalInput")
    d_tpad = nc.dram_tensor("tpad", [P, NS], f32, kind="ExternalInput")
    d_cbias = nc.dram_tensor("cbias", [P, H, NS], f32, kind="ExternalInput")
    d_diag = nc.dram_tensor("diag", [P, P], f32, kind="ExternalInput")
    d_out = nc.dram_tensor("out", [T, D], f32, kind="ExternalOutput")

    with tile.TileContext(nc) as tc, ExitStack() as ctx:
        const = ctx.enter_context(tc.tile_pool(name="const", bufs=1))
        small = ctx.enter_context(tc.tile_pool(name="small", bufs=2))
        psA = ctx.enter_context(tc.tile_pool(name="psA", bufs=2, space="PSUM"))
        psB = ctx.enter_context(tc.tile_pool(name="psB", bufs=2, space="PSUM"))
        psC = ctx.enter_context(tc.tile_pool(name="psC", bufs=2, space="PSUM"))
        psTr = ctx.enter_context(tc.tile_pool(name="psTr", bufs=2, space="PSUM"))

        ident_f = const.tile([P, P], f32)
        make_identity(nc, ident_f)
        ones16 = const.tile([P, 2, P], fp8)
        nc.vector.memset(ones16[:], V_S)
        ones_c = const.tile([P, 2, 1], fp8)
        nc.vector.memset(ones_c[:], 1.0)
        ones_row = const.tile([1, P], bf16)
        nc.vector.memset(ones_row[:], 1.0)
        eps_t = const.tile([P, 1], f32)
        nc.vector.memset(eps_t[:], LN_EPS)
        diag_sb = const.tile([P, P], f32)
        nc.gpsimd.dma_start(out=diag_sb[:], in_=d_diag.ap())
        tpad_sb = const.tile([P, NS], f32)
        nc.gpsimd.dma_start(out=tpad_sb[:], in_=d_tpad.ap())
        cbias_sb = const.tile([P, H, NS], f32)
        nc.gpsimd.dma_start(out=cbias_sb[:], in_=d_cbias.ap())
        bq_sb = const.tile([P, H * ND], f32)
        nc.gpsimd.dma_start(out=bq_sb[:], in_=d_bq.ap())
        b1_sb = const.tile([P, NF], f32)
        nc.gpsimd.dma_start(out=b1_sb[:], in_=d_b1.ap())
        bo_sb = const.tile([1, D], bf16)
        nc.gpsimd.dma_start(out=bo_sb[:], in_=d_bo.ap())
        b2_sb = const.tile([1, D], bf16)
        nc.gpsimd.dma_start(out=b2_sb[:], in_=d_b2.ap())

        def layernorm(res_ap, dst_ap):
            stats = small.tile([P, 6], f32, tag="ln_stats")
            nc.vector.bn_stats(stats[:], res_ap)
            mv = small.tile([P, 2], f32, tag="ln_mv")
            nc.vector.bn_aggr(mv[:], stats[:])
            std = small.tile([P, 1], f32, tag="ln_std")
            nc.scalar.activation(std[:], mv[:, 1:2], AF.Sqrt, bias=eps_t[:])
            istd = small.tile([P, 1], f32, tag="ln_istd")
            nc.vector.reciprocal(istd[:], std[:])
            nc.vector.tensor_scalar(
                out=dst_ap, in0=res_ap, scalar1=mv[:, 0:1], scalar2=istd[:],
                op0=OP.subtract, op1=OP.mult)

        def transpose_cast(src_ap, dst_fn, nblk):
            for k in range(nblk):
                tr_ps = psTr.tile([P, P], f32, tag="tr")
                nc.tensor.transpose(
                    tr_ps[:], src_ap[:, k * P:(k + 1) * P], ident_f[:])
                nc.vector.tensor_copy(dst_fn(k), tr_ps[:])

        es_x2 = ExitStack()
        x2_pool = es_x2.enter_context(tc.tile_pool(name="x2p", bufs=1))
        es_acc = ExitStack()
        acc_pool = es_acc.enter_context(tc.tile_pool(name="accp", bufs=1))
        es_x1 = ExitStack()
        x1_pool = es_x1.enter_context(tc.tile_pool(name="x1p", bufs=1))
        es_x32 = ExitStack()
        x32_pool = es_x32.enter_context(tc.tile_pool(name="x32p", bufs=1))

        x2_sb = x2_pool.tile([P, NT, D], f32, tag="x2")
        x2T = x2_pool.tile([P, ND, T], bf16, tag="x2T")
        acc_sb = acc_pool.tile([P, NT, D], f32, tag="acc")
        x1_sb = x1_pool.tile([P, NT, D], f32, tag="x1")
        x1T8 = x1_pool.tile([P, 2, 2, T], fp8, tag="x1T8")
        x32_sb = x32_pool.tile([P, NT, D], f32)
        for tb in range(NT):
            nc.sync.dma_start(
                out=x32_sb[:, tb, :],
                in_=d_x.ap().rearrange("(tb p) d -> p tb d", p=P)[:, tb, :])

        # ============ phase 1: self attention =========================
        with tc.tile_pool(name="selfp", bufs=1) as sp, \
             tc.tile_pool(name="u8p", bufs=1) as u8p, \
             tc.tile_pool(name="m8str", bufs=2) as m8str, \
             tc.tile_pool(name="rbcp", bufs=2) as rbcp, \
             tc.tile_pool(name="ctxp", bufs=1) as ctxp:
            xT8 = sp.tile([P, 2, 2, T], fp8, tag="xT8")
            xrow8 = sp.tile([P, NS // 2, 2, D], fp8, tag="xrow8")
            q8 = sp.tile([P, 2, 2, T], fp8, tag="q8")
            exp8 = sp.tile([P, NS // 2, 2, T], fp8, tag="exp8")
            nc.gpsimd.memset(exp8[:], 0.0)
            u8_sb = u8p.tile([P, 2, 2, H, D], fp8, tag="u8")
            nc.sync.dma_start(out=u8_sb[:], in_=d_u8.ap())
            ctx8 = []
            for h in range(H):
                ctx8_h = ctxp.tile([P, 2, 2, T], fp8, tag=f"ctx{h}")
                ctx8.append(ctx8_h)

            for tb in range(NT):
                transpose_cast(
                    x32_sb[:, tb, :],
                    lambda k, tb=tb: xT8[:, k // 2, k % 2, tb * P:(tb + 1) * P],
                    ND)
                nc.scalar.activation(
                    xrow8[:, tb // 2, tb % 2, :], x32_sb[:, tb, :], AF.Copy)

            for h in range(H):
                m8_t = m8str.tile([P, 2, 2, ND, P], fp8, tag="m8")
                nc.sync.dma_start(out=m8_t[:], in_=d_m8.ap()[h])
                for eb in range(ND):
                    for c in range(NC2):
                        q_ps = psA.tile([P, 512], f32, tag="a")
                        for ds in range(2):
                            nc.tensor.matmul(
                                q_ps[:], lhsT=m8_t[:, ds, :, eb, :],
                                rhs=xT8[:, ds, :, c * 512:(c + 1) * 512],
                                start=(ds == 0), stop=(ds == 1),
                                perf_mode=PM.DoubleRow)
                        nc.vector.tensor_scalar_add(
                            q8[:, eb // 2, eb % 2, c * 512:(c + 1) * 512],
                            q_ps[:], bq_sb[:, h * ND + eb:h * ND + eb + 1])

                recip_bc = rbcp.tile([P, T], f32, tag="rbc")
                rs_sb = rbcp.tile([P, T], f32, tag="rssb")

                def rowsum_chunk(c):
                    jpmax = 2 * (c + 1)
                    rs_ps = psC.tile([P, 512], f32, tag="c")
                    for jp in range(jpmax):
                        nc.tensor.matmul(
                            rs_ps[:], lhsT=ones16[:],
                            rhs=exp8[:, jp, :, c * 512:(c + 1) * 512],
                            start=(jp == 0), stop=(jp == jpmax - 1),
                            perf_mode=PM.DoubleRow)
                    sl = slice(c * 512, (c + 1) * 512)
                    nc.scalar.activation(rs_sb[:, sl], rs_ps[:], AF.Copy)
                    nc.vector.reciprocal_approx_fast(
                        out=recip_bc[:, sl], in_=rs_sb[:, sl])

                for j in range(NS):
                    c_lo = (j * P) // 512
                    for c in range(c_lo, NC2):
                        lo = max(j * P, c * 512)
                        w = (c + 1) * 512 - lo
                        att_ps = psB.tile([P, 512], f32, tag="b")
                        for ds in range(2):
                            nc.tensor.matmul(
                                att_ps[:, :w],
                                lhsT=xT8[:, ds, :, j * P:(j + 1) * P],
                                rhs=q8[:, ds, :, lo:(c + 1) * 512],
                                start=(ds == 0), stop=(ds == 1),
                                perf_mode=PM.DoubleRow)
                        if lo == j * P:
                            nc.vector.tensor_tensor(
                                out=att_ps[:, 0:P], in0=att_ps[:, 0:P],
                                in1=diag_sb[:], op=OP.add)
                        nc.scalar.activation(
                            exp8[:, j // 2, j % 2, lo:(c + 1) * 512],
                            att_ps[:, :w], AF.Exp,
                            bias=tpad_sb[:, j:j + 1], scale=SCALE / G_M)
                    if j == 3:
                        rowsum_chunk(0)
                rowsum_chunk(1)

                for db in range(ND):
                    for c in range(NC2):
                        jpmax = 2 * (c + 1)
                        ctx_ps = psC.tile([P, 512], f32, tag="c")
                        for jp in range(jpmax):
                            nc.tensor.matmul(
                                ctx_ps[:],
                                lhsT=xrow8[:, jp, :, db * P:(db + 1) * P],
                                rhs=exp8[:, jp, :, c * 512:(c + 1) * 512],
                                start=(jp == 0), stop=(jp == jpmax - 1),
                                perf_mode=PM.DoubleRow)
                        nc.vector.tensor_tensor(
                            out=ctx8[h][:, db // 2, db % 2,
                                        c * 512:(c + 1) * 512],
                            in0=ctx_ps[:],
                            in1=recip_bc[:, c * 512:(c + 1) * 512],
                            op=OP.mult)

            for tb in range(NT):
                o_ps = psA.tile([P, 512], f32, tag="a")
                nc.tensor.matmul(
                    o_ps[:], lhsT=ones_row[:], rhs=bo_sb[:],
                    start=True, stop=False)
                for h in range(H):
                    for es in range(2):
                        nc.tensor.matmul(
                            o_ps[:],
                            lhsT=ctx8[h][:, es, :, tb * P:(tb + 1) * P],
                            rhs=u8_sb[:, es, :, h, :],
                            start=False, stop=(h == H - 1 and es == 1),
                            perf_mode=PM.DoubleRow)
                res = small.tile([P, D], f32, tag="res1")
                nc.vector.scalar_tensor_tensor(
                    out=res[:], in0=o_ps[:], scalar=A_SELF,
                    in1=x32_sb[:, tb, :], op0=OP.mult, op1=OP.add)
                layernorm(res[:], x1_sb[:, tb, :])
                transpose_cast(
                    x1_sb[:, tb, :],
                    lambda k, tb=tb: x1T8[:, k // 2, k % 2,
                                          tb * P:(tb + 1) * P],
                    ND)
        es_x32.close()

        # ============ phase 2: cross attention ========================
        with tc.tile_pool(name="ktstr", bufs=2) as ktstr, \
             tc.tile_pool(name="vtstr", bufs=2) as vtstr, \
             tc.tile_pool(name="expcp", bufs=2) as expcp, \
             tc.tile_pool(name="rcolp", bufs=2) as rcolp:
            for h in range(H):
                kt8_t = ktstr.tile([P, 2, 2, S], fp8, tag="kt8")
                nc.sync.dma_start(out=kt8_t[:], in_=d_kt8.ap()[h])
                vt8_t = vtstr.tile([P, NS // 2, 2, D], fp8, tag="vt8")
                nc.sync.dma_start(out=vt8_t[:], in_=d_vt8.ap()[h])
                for c in range(NC2):
                    exp8c = expcp.tile([P, NS // 2, 2, 512], fp8, tag="expc")
                    for j in range(NS):
                        att_ps = psB.tile([P, 512], f32, tag="b")
                        for ds in range(2):
                            nc.tensor.matmul(
                                att_ps[:],
                                lhsT=kt8_t[:, ds, :, j * P:(j + 1) * P],
                                rhs=x1T8[:, ds, :, c * 512:(c + 1) * 512],
                                start=(ds == 0), stop=(ds == 1),
                                perf_mode=PM.DoubleRow)
                        nc.scalar.activation(
                            exp8c[:, j // 2, j % 2, :], att_ps[:], AF.Exp,
                            bias=cbias_sb[:, h, j:j + 1], scale=SCALE)
                    rsT_t = psC.tile([P, 512], f32, tag="c")
                    rsT_ps = rsT_t[:, 0:4]
                    for tk in range(4):
                        for jp in range(NS // 2):
                            nc.tensor.matmul(
                                rsT_ps[:, tk:tk + 1] if False else rsT_t[:, tk:tk + 1],
                                lhsT=exp8c[:, jp, :, tk * P:(tk + 1) * P],
                                rhs=ones_c[:],
                                start=(jp == 0), stop=(jp == NS // 2 - 1),
                                perf_mode=PM.DoubleRow)
                    rcol = rcolp.tile([P, 4], f32, tag="rcol")
                    nc.vector.reciprocal(rcol[:], rsT_t[:, 0:4])
                    for tk in range(4):
                        ct_ps = psA.tile([P, 512], f32, tag="a")
                        for jp in range(NS // 2):
                            nc.tensor.matmul(
                                ct_ps[:],
                                lhsT=exp8c[:, jp, :, tk * P:(tk + 1) * P],
                                rhs=vt8_t[:, jp, :, :],
                                start=(jp == 0), stop=(jp == NS // 2 - 1),
                                perf_mode=PM.DoubleRow)
                        tb = c * 4 + tk
                        resid = x1_sb if h == 0 else acc_sb
                        nc.vector.scalar_tensor_tensor(
                            out=acc_sb[:, tb, :], in0=ct_ps[:],
                            scalar=rcol[:, tk:tk + 1],
                            in1=resid[:, tb, :], op0=OP.mult, op1=OP.add)
            for tb in range(NT):
                layernorm(acc_sb[:, tb, :], x2_sb[:, tb, :])
                transpose_cast(
                    x2_sb[:, tb, :],
                    lambda k, tb=tb: x2T[:, k, tb * P:(tb + 1) * P],
                    ND)
        es_x1.close()
        es_acc.close()

        # ============ phase 3: FFN ====================================
        with tc.tile_pool(name="ffn", bufs=1) as ffn_pool:
            w1_t = ffn_pool.tile([P, ND, NF, P], bf16, tag="w1")
            nc.sync.dma_start(out=w1_t[:], in_=d_w1.ap().rearrange(
                "(kt p) (fb f) -> p kt fb f", p=P, f=P))
            w2_t = ffn_pool.tile([P, NF, D], bf16, tag="w2")
            nc.sync.dma_start(out=w2_t[:], in_=d_w2.ap().rearrange(
                "(kt p) d -> p kt d", p=P))
            f1T = ffn_pool.tile([P, NF, T], bf16, tag="f1T")

            def f1_chunk(c):
                for fb in range(NF):
                    f_ps = psA.tile([P, 512], f32, tag="a")
                    for kt in range(ND):
                        nc.tensor.matmul(
                            f_ps[:], lhsT=w1_t[:, kt, fb, :],
                            rhs=x2T[:, kt, c * 512:(c + 1) * 512],
                            start=(kt == 0), stop=(kt == ND - 1))
                    nc.scalar.activation(
                        f1T[:, fb, c * 512:(c + 1) * 512], f_ps[:], AF.Relu,
                        bias=b1_sb[:, fb:fb + 1])

            f1_chunk(0)
            f1_chunk(1)
            for tb in range(NT):
                o_ps = psB.tile([P, 512], f32, tag="b")
                for kt in range(NF):
                    nc.tensor.matmul(
                        o_ps[:], lhsT=f1T[:, kt, tb * P:(tb + 1) * P],
                        rhs=w2_t[:, kt, :], start=(kt == 0), stop=False)
                nc.tensor.matmul(
                    o_ps[:], lhsT=ones_row[:], rhs=b2_sb[:],
                    start=False, stop=True)
                res = small.tile([P, D], f32, tag="res3")
                nc.vector.tensor_tensor(
                    out=res[:], in0=o_ps[:], in1=x2_sb[:, tb, :], op=OP.add)
                out_sb = small.tile([P, D], f32, tag="out_sb")
                layernorm(res[:], out_sb[:])
                nc.sync.dma_start(
                    out=d_out.ap().rearrange("(tb p) d -> p tb d", p=P)[:, tb, :],
                    in_=out_sb[:])
        es_x2.close()

    nc.compile()
    _CACHE["nc"] = nc
    return nc


def make_in_maps(inputs):
    import ml_dtypes

    bf = ml_dtypes.bfloat16
    f8 = ml_dtypes.float8_e4m3
    f32 = np.float32

    wq = np.asarray(inputs["Wq_self"], f32)
    wk = np.asarray(inputs["Wk_self"], f32)
    wv = np.asarray(inputs["Wv_self"], f32)
    wo = np.asarray(inputs["Wo_self"], f32)
    wqm = np.asarray(inputs["Wq_mem"], f32)
    wom = np.asarray(inputs["Wo_mem"], f32)
    bq = np.asarray(inputs["bq_self"], f32)
    bk = np.asarray(inputs["bk_self"], f32)  # dropped (softmax shift-invar.)
    bv = np.asarray(inputs["bv_self"], f32)
    bo = np.asarray(inputs["bo_self"], f32)
    bqm = np.asarray(inputs["bq_mem"], f32)
    bom = np.asarray(inputs["bo_mem"], f32)

    # M = Wq@Wk^T, b~ = Wk@bq per head
    m8 = np.empty((H, D, D), f32)
    u8 = np.empty((H, D, D), f32)
    bq_c = np.empty((H, D), f32)
    bo_row = bo.copy()
    for h in range(H):
        m8[h] = wq[h] @ wk[h].T
        u8[h] = wv[h] @ wo[h * D:(h + 1) * D]
        bq_c[h] = wk[h] @ bq[h]
        bo_row += bv[h] @ wo[h * D:(h + 1) * D]

    def col_layout(b_hd):  # [H, D] -> [P, H*ND]
        return np.ascontiguousarray(
            b_hd.reshape(H, ND, P).transpose(2, 0, 1).reshape(P, H * ND)
        ).astype(f32)

    diag = np.asarray(inputs["tgt_subsq_mask"], f32)[:P, :P].T * (G_M / SCALE)

    shared = {
        "m8": np.ascontiguousarray(
            (m8 * G_M).reshape(H, 2, 2, P, ND, P).transpose(0, 3, 1, 2, 4, 5)
        ).astype(f8),
        "u8": np.ascontiguousarray(
            (u8 * G_U).reshape(H, 2, 2, P, D).transpose(3, 1, 2, 0, 4)
        ).astype(f8),
        "w1": np.asarray(inputs["W1"], f32).astype(bf),
        "w2": np.asarray(inputs["W2"], f32).astype(bf),
        "bq_c": col_layout(bq_c * G_M),
        "b1_c": np.ascontiguousarray(
            np.asarray(inputs["b1"], f32).reshape(NF, P).T).astype(f32),
        "bo_row": (bo_row / A_SELF).reshape(1, D).astype(bf),
        "b2_row": np.asarray(inputs["b2"], f32).reshape(1, D).astype(bf),
        "diag": np.ascontiguousarray(diag).astype(f32),
    }

    memk = np.asarray(inputs["mem_keys"], f32)
    memv = np.asarray(inputs["mem_values"], f32)
    x = np.asarray(inputs["x"], f32)
    spad = np.asarray(inputs["src_padding_mask"], f32)[:, :, 0]
    tpad = np.asarray(inputs["tgt_padding_mask"], f32)[:, :, 0]

    in_maps = []
    for b in range(B):
        kt8 = np.empty((H, D, S), f32)
        vt8 = np.empty((H, S, D), f32)
        cb = np.empty((H, S), f32)
        for h in range(H):
            kt8[h] = wqm[h] @ memk[b].T
            vt8[h] = memv[b] @ wom[h * D:(h + 1) * D] + bom / H
            cb[h] = SCALE * (memk[b] @ bqm[h]) + spad[b]
        m = dict(shared)
        m["x32"] = np.ascontiguousarray(x[b])
        m["kt8"] = np.ascontiguousarray(
            kt8.reshape(H, 2, 2, P, S).transpose(0, 3, 1, 2, 4)).astype(f8)
        m["vt8"] = np.ascontiguousarray(
            vt8.reshape(H, NS // 2, 2, P, D).transpose(0, 3, 1, 2, 4)).astype(f8)
        # cbias [P, H, NS]: col (h, j) = bias for s-block j
        m["cbias"] = np.ascontiguousarray(
            cb.reshape(H, NS, P).transpose(2, 0, 1)).astype(f32)
        m["tpad"] = np.ascontiguousarray(tpad[b].reshape(NS, P).T)
        in_maps.append(m)
    return in_maps


def kernel(**inputs):
    from concourse.bass_utils import run_bass_kernel_spmd

    nc = _build()
    in_maps = make_in_maps(inputs)
    res = run_bass_kernel_spmd(nc, in_maps, list(range(B)))
    out = np.stack([np.asarray(res.results[i]["out"]) for i in range(B)])
    return out.astype(np.float32)
